# revision 1
# baseline (speedup 1.0000x reference)
"""Rational-quadratic spline (neural spline flow) forward kernel for TRN2.

Strategy (no per-lane gather exists on TRN2, so everything is computed by
telescoped compare-accumulate over the 29 interior knots):

  - Data-parallel over 8 NeuronCores: batch rows sharded (padded to 63488/core).
  - On-chip layout: transpose 128x128 blocks with the PE so each partition
    holds one (batch-phase, variable) pair; all per-variable constants become
    per-partition scalars.
  - Per element x (per lane):
        S(x) = S_0[v] + sum_k (x >= cw[v,k]) * dS[v,k]
    for six streams S in {cw, ch, AM, BM, AD, BD} via a custom DVE op
    (out = in1 + (in0 >= s0)*s1; s0/s1 per-partition scalars).
  - Spline evaluated as  out = ch + M/D,  M = (AM*tt + BM)*tt,
    D = (AD*tt + BD)*tt + 1,  tt = x - cw;  the division runs on the ACT
    engine as exp(log M - log D) (DVE reciprocal is slow, ACT recip banned).
  - logabsdet = log(M'D - MD') - 2 log D with M' = 2*AM*tt + BM etc.
  - Outside [-5,5]: select(x) / select(0) via a fused custom select op.
"""

import numpy as np

TAIL_BOUND = 5.0
MIN_BIN_WIDTH = 1e-3
MIN_BIN_HEIGHT = 1e-3
MIN_DERIVATIVE = 1e-3
K = 30
V = 16
NCORES = 8

# fixed problem shape (self-contained; harness calls with B=500000)
_LANES = 128
_BLK = 128


# --------------------------------------------------------------------------- #
# Custom DVE ops (registered once at import)
# --------------------------------------------------------------------------- #
_OPS_REGISTERED = {}


def _register_custom_ops():
    if _OPS_REGISTERED:
        return _OPS_REGISTERED
    import concourse.dve_ops as dve_ops
    from concourse.dve_ops import DveOp, get_dve_sub_opcode, has_src1
    from concourse.dve_spec import (
        Spec, Src0, Src1, C0, C1, C3, Zero, select, lower, _spill_c3_to_src1,
    )
    from concourse.dve_uop import DveOpSpec

    def mk(name, spec):
        # compute pinned sha for both vers
        sha = {}
        for ver in ("v3", "v4"):
            compiled = DveOpSpec(
                name=name, uops=lower(spec, ver=ver), rd1_en=has_src1(spec)
            )
            sha[ver] = compiled.sha(ver)
        op = DveOp(name, spec, subdim=False, uops_sha=sha)
        dve_ops.OPS.append(op)
        dve_ops.CUSTOM_DVE_SPECS[op.name] = op.spec
        dve_ops._SUB_OPCODE_FOR_NAME[op.name] = (
            dve_ops._CUSTOM_DVE_ROW_BASE + len(dve_ops.OPS) - 1
        )
        assert dve_ops._SUB_OPCODE_FOR_NAME[op.name] < 0x20
        return op

    STEP = mk(
        "RQS_STEP_ANT",
        Spec(
            body=Src1 + (Src0 >= C0) * C1,
            reference=lambda in0, in1, s0, s1, imm2: (
                in1 + (in0 >= s0).astype(np.float32) * s1
            ),
        ),
    )
    INIT = mk(
        "RQS_INIT_ANT",
        Spec(
            body=_spill_c3_to_src1((Src0 >= C0) * C1 + C3),
            reference=lambda in0, in1, s0, s1, imm2: (
                (in0 >= s0).astype(np.float32) * s1 + in1.reshape(-1, 1)
            ),
        ),
    )
    SEL_X = mk(
        "RQS_SEL_X_ANT",
        Spec(
            body=select((Src0 >= C0) & (Src0 <= C1), Src1, Src0),
            reference=lambda in0, in1, s0, s1, imm2: np.where(
                (in0 >= s0) & (in0 <= s1), in1, in0
            ).astype(np.float32),
        ),
    )
    SEL_0 = mk(
        "RQS_SEL_0_ANT",
        Spec(
            body=select((Src0 >= C0) & (Src0 <= C1), Src1, Zero),
            reference=lambda in0, in1, s0, s1, imm2: np.where(
                (in0 >= s0) & (in0 <= s1), in1, 0.0
            ).astype(np.float32),
        ),
    )
    _OPS_REGISTERED.update(STEP=STEP, INIT=INIT, SEL_X=SEL_X, SEL_0=SEL_0)
    return _OPS_REGISTERED


# --------------------------------------------------------------------------- #
# Host-side table construction
# --------------------------------------------------------------------------- #
def _softmax(x, axis=-1):
    x = x - x.max(axis=axis, keepdims=True)
    e = np.exp(x)
    return e / e.sum(axis=axis, keepdims=True)


def _softplus(x):
    return np.log1p(np.exp(-np.abs(x))) + np.maximum(x, 0)


def _knots(unnorm, min_bin, lo, hi):
    w = _softmax(unnorm.astype(np.float64), axis=-1)
    w = min_bin + (1.0 - min_bin * K) * w
    cw = np.cumsum(w, axis=-1)
    cw = np.pad(cw, ((0, 0), (1, 0)))
    cw = (hi - lo) * cw + lo
    cw[..., 0] = lo
    cw[..., -1] = hi
    return cw  # (V, K+1)


def _build_tables(uw, uh, ud):
    lo, hi = -TAIL_BOUND, TAIL_BOUND
    const = np.log(np.exp(1.0 - MIN_DERIVATIVE) - 1.0)
    udp = np.concatenate(
        [np.full((V, 1), const), ud.astype(np.float64), np.full((V, 1), const)],
        axis=-1,
    )
    d = MIN_DERIVATIVE + _softplus(udp)  # (V,K+1)

    cw = _knots(uw, MIN_BIN_WIDTH, lo, hi)
    chts = _knots(uh, MIN_BIN_HEIGHT, lo, hi)

    w = cw[:, 1:] - cw[:, :-1]
    h = chts[:, 1:] - chts[:, :-1]
    delta = h / w
    a = 1.0 / w
    dk = d[:, :-1]
    dk1 = d[:, 1:]

    AM = h * a * a * (1.0 - dk / delta)
    BM = h * a * dk / delta
    gam = (dk + dk1 - 2.0 * delta) / delta
    AD = -gam * a * a
    BD = gam * a

    streams = [cw[:, :-1], chts[:, :-1], AM, BM, AD, BD]  # each (V,K)
    thr = cw[:, 1:K]  # (V,29)

    # consts table, one row per partition p (variable v = p % 16):
    # cols 0..28: thresholds; then per stream: [base, 29 deltas] = 30 cols
    NC = 29 + 6 * 30
    consts = np.zeros((_LANES, NC), dtype=np.float32)
    vidx = np.arange(_LANES) % V
    consts[:, 0:29] = thr[vidx].astype(np.float32)
    for si, S in enumerate(streams):
        base = S[:, 0]
        dS = np.diff(S, axis=1)  # (V,29)
        consts[:, 29 + si * 30] = base[vidx].astype(np.float32)
        consts[:, 29 + si * 30 + 1 : 29 + (si + 1) * 30] = dS[vidx].astype(
            np.float32
        )
    return consts


# --------------------------------------------------------------------------- #
# Bass program
# --------------------------------------------------------------------------- #
_PROGRAM_CACHE = {}


def _build_program(R, tiles):
    """R: x8-rows per core; tiles: list of free-sizes (multiples of 128)."""
    key = (R, tuple(tiles))
    if key in _PROGRAM_CACHE:
        return _PROGRAM_CACHE[key]

    import concourse.bass as bass
    import concourse.bacc as bacc
    import concourse.tile as tile
    from concourse import mybir
    from concourse._compat import axon_active

    ops = _register_custom_ops()
    STEP, INIT, SEL_X, SEL_0 = ops["STEP"], ops["INIT"], ops["SEL_X"], ops["SEL_0"]

    f32 = mybir.dt.float32
    NC = 29 + 6 * 30
    ALU = mybir.AluOpType
    AF = mybir.ActivationFunctionType

    nc = bacc.Bacc(
        "TRN2",
        target_bir_lowering=False,
        debug=False,
        num_devices=NCORES,
    )
    x_d = nc.dram_tensor("x", (R, 128), f32, kind="ExternalInput")
    c_d = nc.dram_tensor("consts", (_LANES, NC), f32, kind="ExternalInput")
    i_d = nc.dram_tensor("ident", (_LANES, _LANES), f32, kind="ExternalInput")
    o_d = nc.dram_tensor("out", (R, 128), f32, kind="ExternalOutput")
    l_d = nc.dram_tensor("lad", (R, 128), f32, kind="ExternalOutput")

    x_ap, c_ap, i_ap = x_d.ap(), c_d.ap(), i_d.ap()
    o_ap, l_ap = o_d.ap(), l_d.ap()

    with tile.TileContext(nc) as tc:
        from contextlib import ExitStack

        with ExitStack() as ctx:
            cpool = ctx.enter_context(tc.tile_pool(name="const", bufs=1))
            consts = cpool.tile([_LANES, NC], f32)
            nc.sync.dma_start(consts[:], c_ap)
            ident = cpool.tile([_LANES, _LANES], f32)
            nc.sync.dma_start(ident[:], i_ap)

            xpool = ctx.enter_context(tc.tile_pool(name="xin", bufs=2))
            xtpool = ctx.enter_context(tc.tile_pool(name="xt", bufs=2))
            apool = ctx.enter_context(tc.tile_pool(name="acc", bufs=1))
            tpool = ctx.enter_context(tc.tile_pool(name="tmp", bufs=1))
            opool = ctx.enter_context(tc.tile_pool(name="outs", bufs=2))
            psin = ctx.enter_context(
                tc.tile_pool(name="psin", bufs=2, space="PSUM")
            )
            psout = ctx.enter_context(
                tc.tile_pool(name="psout", bufs=1, space="PSUM")
            )
            pacc = ctx.enter_context(
                tc.tile_pool(name="pacc", bufs=1, space="PSUM")
            )

            def thr_ap(k):  # k in 1..29
                return consts[:, k - 1 : k]

            def base_ap(si):
                return consts[:, 29 + si * 30 : 29 + si * 30 + 1]

            def dlt_ap(si, k):  # k in 1..29
                c = 29 + si * 30 + k
                return consts[:, c : c + 1]

            r0 = 0
            for F in tiles:
                nb = F // _BLK
                X = xpool.tile([_LANES, F], f32, tag="X")
                src = x_ap[r0 : r0 + F, :].rearrange("(b p) f -> p b f", p=128)
                dst3 = X[:].rearrange("p (b f) -> p b f", b=nb)
                nc.sync.dma_start(dst3, src)

                XT = xtpool.tile([_LANES, F], f32, tag="XT")
                for b in range(nb):
                    pt = psin.tile([_LANES, _BLK], f32, tag="pt")
                    nc.tensor.transpose(
                        pt[:], X[:, b * _BLK : (b + 1) * _BLK], ident[:]
                    )
                    nc.scalar.copy(XT[:, b * _BLK : (b + 1) * _BLK], pt[:])

                accs = []
                for si in range(2):
                    acc = apool.tile([_LANES, F], f32, tag=f"acc{si}")
                    nc.vector._custom_dve(
                        INIT,
                        out=acc[:],
                        in0=XT[:],
                        in1=base_ap(si),
                        s0=thr_ap(1),
                        s1=dlt_ap(si, 1),
                    )
                    for k in range(2, 30):
                        nc.vector._custom_dve(
                            STEP,
                            out=acc[:],
                            in0=XT[:],
                            in1=acc[:],
                            s0=thr_ap(k),
                            s1=dlt_ap(si, k),
                        )
                    accs.append(acc)
                # streams 2,3: DVE makes (x>=thr)*dlt masks (2x-rate TS);
                # the PE accumulates them into PSUM via identity matmuls;
                # ACT copies out adding the stream base via its bias port.
                for si in (2, 3):
                    accP = pacc.tile(
                        [_LANES, F], f32, tag=f"accP{si}", name=f"accP{si}"
                    )
                    for k in range(1, 30):
                        m = tpool.tile(
                            [_LANES, F], f32, tag=f"maskC{si}_{k % 3}",
                            name=f"maskC{si}_{k % 3}",
                        )
                        nc.vector.tensor_scalar(
                            m[:], XT[:], thr_ap(k), dlt_ap(si, k),
                            op0=ALU.is_ge, op1=ALU.mult,
                        )
                        for h0 in range(0, F, 512):
                            h1 = min(h0 + 512, F)
                            nc.tensor.matmul(
                                accP[:, h0:h1], ident[:], m[:, h0:h1],
                                start=(k == 1), stop=(k == 29),
                            )
                    acc = apool.tile([_LANES, F], f32, tag=f"acc{si}")
                    nc.scalar.activation(
                        acc[:], accP[:], AF.Identity, bias=base_ap(si)
                    )
                    accs.append(acc)
                # streams 4,5: DVE computes (x>=thr)*dlt masks (dual-op
                # tensor_scalar, 2x mode); the otherwise-idle Pool engine
                # accumulates them.
                for si in (4, 5):
                    acc = apool.tile([_LANES, F], f32, tag=f"acc{si}")
                    nc.gpsimd.tensor_scalar(
                        acc[:], XT[:], 0.0, base_ap(si), op0=ALU.mult, op1=ALU.add
                    )
                    for k in range(1, 30):
                        m = tpool.tile(
                            [_LANES, F], f32, tag=f"mask{si}_{k % 2}",
                            name=f"mask{si}_{k % 2}",
                        )
                        nc.vector.tensor_scalar(
                            m[:], XT[:], thr_ap(k), dlt_ap(si, k),
                            op0=ALU.is_ge, op1=ALU.mult,
                        )
                        nc.gpsimd.tensor_tensor(acc[:], acc[:], m[:], op=ALU.add)
                    accs.append(acc)
                cw, ch, AM, BM, AD, BD = accs

                def tmp(name):
                    return tpool.tile([_LANES, F], f32, tag=name, name=name)

                tt = tmp("tt")
                nc.vector.tensor_tensor(tt[:], XT[:], cw[:], op=ALU.subtract)
                t1 = tmp("t1")
                nc.vector.tensor_tensor(t1[:], AM[:], tt[:], op=ALU.mult)
                t3 = tmp("t3")
                nc.vector.tensor_tensor(t3[:], t1[:], BM[:], op=ALU.add)
                M = tmp("M")
                nc.vector.tensor_tensor(M[:], t3[:], tt[:], op=ALU.mult)
                # clamp: exact knot hits give M == 0 -> Ln(0) NaN/-inf on ACT
                nc.vector.tensor_scalar(M[:], M[:], 1e-30, None, op0=ALU.max)
                t4 = tmp("t4")
                nc.vector.tensor_tensor(t4[:], AD[:], tt[:], op=ALU.mult)
                t5 = tmp("t5")
                nc.vector.tensor_tensor(t5[:], t4[:], BD[:], op=ALU.add)
                q = tmp("q")
                nc.vector.tensor_tensor(q[:], t5[:], tt[:], op=ALU.mult)
                LM = tmp("LM")
                nc.scalar.activation(LM[:], M[:], AF.Ln)
                LD = tmp("LD")
                nc.scalar.activation(LD[:], q[:], AF.Ln, bias=1.0)
                s = tmp("s")
                nc.gpsimd.tensor_tensor(s[:], LM[:], LD[:], op=ALU.subtract)
                E = tmp("E")
                nc.scalar.activation(E[:], s[:], AF.Exp)
                outsp = tmp("outsp")
                nc.vector.tensor_tensor(outsp[:], ch[:], E[:], op=ALU.add)
                Mp = tmp("Mp")
                nc.gpsimd.tensor_tensor(Mp[:], t1[:], t3[:], op=ALU.add)
                Dp = tmp("Dp")
                nc.gpsimd.tensor_tensor(Dp[:], t4[:], t5[:], op=ALU.add)
                D = tmp("D")
                nc.vector.tensor_scalar(
                    D[:], q[:], 1.0, None, op0=ALU.add
                )
                u1 = tmp("u1")
                nc.gpsimd.tensor_tensor(u1[:], Mp[:], D[:], op=ALU.mult)
                u2 = tmp("u2")
                nc.gpsimd.tensor_tensor(u2[:], M[:], Dp[:], op=ALU.mult)
                P = tmp("P")
                nc.gpsimd.tensor_tensor(P[:], u1[:], u2[:], op=ALU.subtract)
                LP = tmp("LP")
                nc.scalar.activation(LP[:], P[:], AF.Ln)
                l1 = tmp("l1")
                nc.gpsimd.tensor_tensor(l1[:], LP[:], LD[:], op=ALU.subtract)
                lad0 = tmp("lad0")
                nc.gpsimd.tensor_tensor(lad0[:], l1[:], LD[:], op=ALU.subtract)

                outs_f = opool.tile([_LANES, F], f32, tag="outs_f")
                nc.vector._custom_dve(
                    SEL_X,
                    out=outs_f[:],
                    in0=XT[:],
                    in1=outsp[:],
                    s0=-TAIL_BOUND,
                    s1=TAIL_BOUND,
                )
                lad_f = opool.tile([_LANES, F], f32, tag="lad_f")
                nc.vector._custom_dve(
                    SEL_0,
                    out=lad_f[:],
                    in0=XT[:],
                    in1=lad0[:],
                    s0=-TAIL_BOUND,
                    s1=TAIL_BOUND,
                )

                outT = opool.tile([_LANES, F], f32, tag="outT")
                ladT = opool.tile([_LANES, F], f32, tag="ladT")
                for b in range(nb):
                    po = psout.tile([_LANES, _BLK], f32, tag="po")
                    nc.tensor.transpose(
                        po[:], outs_f[:, b * _BLK : (b + 1) * _BLK], ident[:]
                    )
                    nc.scalar.copy(outT[:, b * _BLK : (b + 1) * _BLK], po[:])
                    pl = psout.tile([_LANES, _BLK], f32, tag="pl")
                    nc.tensor.transpose(
                        pl[:], lad_f[:, b * _BLK : (b + 1) * _BLK], ident[:]
                    )
                    nc.scalar.copy(ladT[:, b * _BLK : (b + 1) * _BLK], pl[:])
                dsto = o_ap[r0 : r0 + F, :].rearrange("(b p) f -> p b f", p=128)
                nc.sync.dma_start(dsto, outT[:].rearrange("p (b f) -> p b f", b=nb))
                dstl = l_ap[r0 : r0 + F, :].rearrange("(b p) f -> p b f", p=128)
                nc.sync.dma_start(dstl, ladT[:].rearrange("p (b f) -> p b f", b=nb))

                r0 += F

    nc.compile()
    _PROGRAM_CACHE[key] = nc
    return nc


# --------------------------------------------------------------------------- #
# Entry point
# --------------------------------------------------------------------------- #
def kernel(inputs, unnormalized_widths, unnormalized_heights,
           unnormalized_derivatives):
    inputs = np.asarray(inputs, dtype=np.float32)
    uw = np.asarray(unnormalized_widths, dtype=np.float32)
    uh = np.asarray(unnormalized_heights, dtype=np.float32)
    ud = np.asarray(unnormalized_derivatives, dtype=np.float32)

    B = inputs.shape[0]
    consts = _build_tables(uw, uh, ud)
    ident = np.eye(_LANES, dtype=np.float32)

    # pad B so each core gets rows divisible by 8*128 (one 128x128 block = 1024 rows)
    rows_per_core = -(-B // NCORES)
    rows_per_core = ((rows_per_core + 1023) // 1024) * 1024
    Bp = rows_per_core * NCORES
    xp = np.zeros((Bp, V), dtype=np.float32)
    xp[:B] = inputs

    R = rows_per_core * V // 128  # x8-rows per core
    # tiles of free-size (multiples of 128), at most 1024
    nblk = R // _BLK
    tiles = []
    while nblk > 0:
        t = min(8, nblk)
        tiles.append(t * _BLK)
        nblk -= t

    nc = _build_program(R, tiles)

    from concourse.bass_utils import run_bass_kernel_spmd

    in_maps = []
    for c in range(NCORES):
        xc = xp[c * rows_per_core : (c + 1) * rows_per_core].reshape(R, 128)
        in_maps.append({"x": xc, "consts": consts, "ident": ident})

    res = run_bass_kernel_spmd(nc, in_maps, core_ids=list(range(NCORES)))

    outs = np.empty((Bp, V), dtype=np.float32)
    lads = np.empty((Bp, V), dtype=np.float32)
    for c in range(NCORES):
        r = res.results[c]
        outs[c * rows_per_core : (c + 1) * rows_per_core] = r["out"].reshape(
            rows_per_core, V
        )
        lads[c * rows_per_core : (c + 1) * rows_per_core] = r["lad"].reshape(
            rows_per_core, V
        )
    return outs[:B], lads[:B]


def run_traced(inputs_dict):
    """Run once with NTFF tracing; returns HW exec time in ns (or None)."""
    inputs = np.asarray(inputs_dict["inputs"], dtype=np.float32)
    uw = np.asarray(inputs_dict["unnormalized_widths"], dtype=np.float32)
    uh = np.asarray(inputs_dict["unnormalized_heights"], dtype=np.float32)
    ud = np.asarray(inputs_dict["unnormalized_derivatives"], dtype=np.float32)
    B = inputs.shape[0]
    consts = _build_tables(uw, uh, ud)
    ident = np.eye(_LANES, dtype=np.float32)
    rows_per_core = ((-(-B // NCORES) + 1023) // 1024) * 1024
    Bp = rows_per_core * NCORES
    xp = np.zeros((Bp, V), dtype=np.float32)
    xp[:B] = inputs
    R = rows_per_core * V // 128
    nblk = R // _BLK
    tiles = []
    while nblk > 0:
        t = min(8, nblk)
        tiles.append(t * _BLK)
        nblk -= t
    nc = _build_program(R, tiles)
    from concourse.bass_utils import run_bass_kernel_spmd

    in_maps = []
    for c in range(NCORES):
        xc = xp[c * rows_per_core : (c + 1) * rows_per_core].reshape(R, 128)
        in_maps.append({"x": xc, "consts": consts, "ident": ident})
    res = run_bass_kernel_spmd(
        nc, in_maps, core_ids=list(range(NCORES)), trace=True
    )
    return res.exec_time_ns


if __name__ == "__main__":
    B = 4096
    rng = np.random.default_rng(0)
    x = rng.standard_normal((B, V)).astype(np.float32)
    uw = rng.random((V, K), dtype=np.float32)
    uh = rng.random((V, K), dtype=np.float32)
    ud = rng.random((V, K - 1), dtype=np.float32)
    o, l = kernel(x, uw, uh, ud)
    print("kernel ran", o.shape, l.shape)



# revision 9
# speedup vs baseline: 2.2672x; 2.2672x over previous
"""Rational-quadratic spline (neural spline flow) forward kernel for TRN2.

Architecture (v2 — "knots on partitions" one-hot/step matmul):

  - Data-parallel over 8 NeuronCores, batch rows sharded (62720 rows/core).
  - Per chunk of 14 row-blocks (1792 rows x 16 vars = 28672 elements):
      1. DMA x in element-major [128, (block, var)].
      2. PE transposes row-block PAIRS [128, 32] -> XT PSUM [32, pair*128]
         (f32r, exact), one DVE/ACT copy escapes XT to SBUF.
      3. PE "replication" matmuls R_gp^T @ XTS -> XB [120, cols] per
         (4-var group g, block parity): partition r = 4k+c holds x of var
         4g+c replicated over the 30 knot rows k.
      4. One compare op per (g, parity) produces ALL 29 step masks at once
         (DVE is_ge -> {0,1}, or ACT Sign -> {-1,1}); knot row k=29 has
         threshold -1e30 == always-on and carries the stream base.
      5. One small transpose-matmul per (block, group): M^T @ T_g -> E
         [elements, 4 vars x 6 streams] in PSUM: all six telescoped
         stream sums {cw, ch, AM, BM, AD, BD} per element in one shot.
      6. Rational-quadratic formula element-major across DVE/Pool/ACT;
         division via exp(ln M - ln D); outside [-5,5] select(x)/select(0).
"""

import numpy as np

TAIL_BOUND = 5.0
MIN_BIN_WIDTH = 1e-3
MIN_BIN_HEIGHT = 1e-3
MIN_DERIVATIVE = 1e-3
K = 30
V = 16
NCORES = 8

_LANES = 128
CH = 16                 # row-blocks per chunk
ROWS_CHUNK = CH * 128   # 2048
PACK = 128              # E columns per block (4 groups x 24, padded to a
                        # quarter PSUM bank so no matmul straddles a bank)
NGRP = 4                # 4-variable groups
GW = 120                # partitions per XB/mask tile (30 knots x 4 vars)
# mask producer per (g, parity, half): index = (g*2+par)*2+h; True -> DVE is_ge
_MASK_ON_DVE = [True, False, False, True, False, True, False, False] * 2


# --------------------------------------------------------------------------- #
# Custom DVE ops
# --------------------------------------------------------------------------- #
_OPS_REGISTERED = {}


def _register_custom_ops():
    if _OPS_REGISTERED:
        return _OPS_REGISTERED
    import concourse.dve_ops as dve_ops
    from concourse.dve_ops import DveOp, has_src1
    from concourse.dve_spec import Spec, Src0, Src1, C0, C1, Zero, select, maxx, lower
    from concourse.dve_uop import DveOpSpec

    def mk(name, spec):
        sha = {}
        for ver in ("v3", "v4"):
            compiled = DveOpSpec(
                name=name, uops=lower(spec, ver=ver), rd1_en=has_src1(spec)
            )
            sha[ver] = compiled.sha(ver)
        op = DveOp(name, spec, subdim=False, uops_sha=sha)
        dve_ops.OPS.append(op)
        dve_ops.CUSTOM_DVE_SPECS[op.name] = op.spec
        dve_ops._SUB_OPCODE_FOR_NAME[op.name] = (
            dve_ops._CUSTOM_DVE_ROW_BASE + len(dve_ops.OPS) - 1
        )
        assert dve_ops._SUB_OPCODE_FOR_NAME[op.name] < 0x20
        return op

    MULMAX = mk(
        "RQS2_MULMAX_ANT",
        Spec(
            body=maxx(Src0 * Src1, C0),
            reference=lambda in0, in1, s0, s1, imm2: np.maximum(
                in0 * in1, s0
            ).astype(np.float32),
        ),
    )
    SUBSUB = mk(
        "RQS2_SUBSUB_ANT",
        Spec(
            body=(Src0 - Src1) - Src1,
            reference=lambda in0, in1, s0, s1, imm2: (in0 - 2.0 * in1).astype(
                np.float32
            ),
        ),
    )
    SEL_X = mk(
        "RQS2_SEL_X_ANT",
        Spec(
            body=select((Src0 >= C0) & (Src0 <= C1), Src1, Src0),
            reference=lambda in0, in1, s0, s1, imm2: np.where(
                (in0 >= s0) & (in0 <= s1), in1, in0
            ).astype(np.float32),
        ),
    )
    SEL_0 = mk(
        "RQS2_SEL_0_ANT",
        Spec(
            body=select((Src0 >= C0) & (Src0 <= C1), Src1, Zero),
            reference=lambda in0, in1, s0, s1, imm2: np.where(
                (in0 >= s0) & (in0 <= s1), in1, 0.0
            ).astype(np.float32),
        ),
    )
    _OPS_REGISTERED.update(MULMAX=MULMAX, SUBSUB=SUBSUB, SEL_X=SEL_X, SEL_0=SEL_0)
    return _OPS_REGISTERED


# --------------------------------------------------------------------------- #
# Host-side table construction
# --------------------------------------------------------------------------- #
def _softmax(x, axis=-1):
    x = x - x.max(axis=axis, keepdims=True)
    e = np.exp(x)
    return e / e.sum(axis=axis, keepdims=True)


def _softplus(x):
    return np.log1p(np.exp(-np.abs(x))) + np.maximum(x, 0)


def _knots(unnorm, min_bin, lo, hi):
    w = _softmax(unnorm.astype(np.float64), axis=-1)
    w = min_bin + (1.0 - min_bin * K) * w
    cw = np.cumsum(w, axis=-1)
    cw = np.pad(cw, ((0, 0), (1, 0)))
    cw = (hi - lo) * cw + lo
    cw[..., 0] = lo
    cw[..., -1] = hi
    return cw  # (V, K+1)


def _build_tables(uw, uh, ud):
    """Returns THR (120,4), NTHR (120,4), R (64, 8*120), T (120, 16*24)."""
    lo, hi = -TAIL_BOUND, TAIL_BOUND
    const = np.log(np.exp(1.0 - MIN_DERIVATIVE) - 1.0)
    udp = np.concatenate(
        [np.full((V, 1), const), ud.astype(np.float64), np.full((V, 1), const)],
        axis=-1,
    )
    d = MIN_DERIVATIVE + _softplus(udp)  # (V,K+1)

    cw = _knots(uw, MIN_BIN_WIDTH, lo, hi)
    chts = _knots(uh, MIN_BIN_HEIGHT, lo, hi)

    w = cw[:, 1:] - cw[:, :-1]
    h = chts[:, 1:] - chts[:, :-1]
    delta = h / w
    a = 1.0 / w
    dk = d[:, :-1]
    dk1 = d[:, 1:]

    AM = h * a * a * (1.0 - dk / delta)
    BM = h * a * dk / delta
    gam = (dk + dk1 - 2.0 * delta) / delta
    AD = -gam * a * a
    BD = gam * a

    streams = [cw[:, :-1], chts[:, :-1], AM, BM, AD, BD]  # each (V,K)
    thr = cw[:, 1:K]  # (V,29) interior knots

    THR = np.full((GW, NGRP), -1e30, dtype=np.float32)
    T = np.zeros((GW, 16 * 24), dtype=np.float32)
    Rm = np.zeros((64, 8 * GW), dtype=np.float32)
    for g in range(NGRP):
        for par in range(2):
            gp = g * 2 + par
            for c in range(4):
                v = 4 * g + c
                Rm[16 * par + v, gp * GW + np.arange(30) * 4 + c] = 1.0
                Rm[32 + 16 * par + v, gp * GW + np.arange(30) * 4 + c] = 1.0
                if par == 0:
                    for k in range(29):
                        THR[4 * k + c, g] = thr[v, k]
            for h in range(2):
                gph = gp * 2 + h
                on_dve = _MASK_ON_DVE[gph]
                for c in range(4):
                    v = 4 * g + c
                    for si, S in enumerate(streams):
                        dS = np.diff(S[v])  # (29,)
                        base = S[v, 0]
                        col = gph * 24 + c * 6 + si
                        if on_dve:
                            T[4 * np.arange(29) + c, col] = dS.astype(np.float32)
                            T[4 * 29 + c, col] = np.float32(base)
                        else:
                            T[4 * np.arange(29) + c, col] = (dS / 2.0).astype(
                                np.float32
                            )
                            T[4 * 29 + c, col] = np.float32(
                                base + dS.sum() / 2.0
                            )
    NTHR = (-THR).astype(np.float32)
    return THR, NTHR, Rm, T


# --------------------------------------------------------------------------- #
# Bass program
# --------------------------------------------------------------------------- #
_PROGRAM_CACHE = {}


def _build_program(rows_per_core):
    key = rows_per_core
    if key in _PROGRAM_CACHE:
        return _PROGRAM_CACHE[key]

    import concourse.bass as bass
    import concourse.bacc as bacc
    import concourse.tile as tile
    from concourse import mybir
    from contextlib import ExitStack

    ops = _register_custom_ops()
    MULMAX, SUBSUB = ops["MULMAX"], ops["SUBSUB"]
    SEL_X, SEL_0 = ops["SEL_X"], ops["SEL_0"]

    f32 = mybir.dt.float32
    f32r = mybir.dt.float32r
    bf16 = mybir.dt.bfloat16
    ALU = mybir.AluOpType
    AF = mybir.ActivationFunctionType

    assert rows_per_core % ROWS_CHUNK == 0
    nchunks = rows_per_core // ROWS_CHUNK
    FE = CH * 16          # element-major free size per chunk (224)
    NPAIR = CH // 2       # 7
    XTW = NPAIR * 128     # 896 cols per parity

    nc = bacc.Bacc(
        "TRN2", target_bir_lowering=False, debug=False, num_devices=NCORES
    )
    x_d = nc.dram_tensor("x", (rows_per_core, 16), f32, kind="ExternalInput")
    thr_d = nc.dram_tensor("thr", (GW, NGRP), f32, kind="ExternalInput")
    nthr_d = nc.dram_tensor("nthr", (GW, NGRP), f32, kind="ExternalInput")
    r_d = nc.dram_tensor("rmat", (64, 8 * GW), bf16, kind="ExternalInput")
    t_d = nc.dram_tensor("tbl", (GW, 16 * 24), f32, kind="ExternalInput")
    i_d = nc.dram_tensor("ident", (_LANES, _LANES), bf16, kind="ExternalInput")
    o_d = nc.dram_tensor("out", (rows_per_core, 16), f32, kind="ExternalOutput")
    l_d = nc.dram_tensor("lad", (rows_per_core, 16), f32, kind="ExternalOutput")

    x_ap, o_ap, l_ap = x_d.ap(), o_d.ap(), l_d.ap()

    with tile.TileContext(nc) as tc:
        with ExitStack() as ctx:
            cpool = ctx.enter_context(tc.tile_pool(name="const", bufs=1))
            THR = cpool.tile([GW, NGRP], f32)
            nc.sync.dma_start(THR[:], thr_d.ap())
            NTHR = cpool.tile([GW, NGRP], f32)
            nc.sync.dma_start(NTHR[:], nthr_d.ap())
            RM = cpool.tile([64, 8 * GW], bf16)
            nc.sync.dma_start(RM[:], r_d.ap())
            TT = cpool.tile([GW, 16 * 24], f32)
            nc.sync.dma_start(TT[:], t_d.ap())
            IDT = cpool.tile([_LANES, _LANES], bf16)
            nc.sync.dma_start(IDT[:], i_d.ap())

            xpool = ctx.enter_context(tc.tile_pool(name="xin", bufs=2))
            xts_pool = ctx.enter_context(tc.tile_pool(name="xts", bufs=2))
            mpool = ctx.enter_context(tc.tile_pool(name="masks", bufs=2))
            tpool = ctx.enter_context(tc.tile_pool(name="tmp", bufs=1))
            opool = ctx.enter_context(tc.tile_pool(name="outs", bufs=2))
            ps_xt = ctx.enter_context(
                tc.tile_pool(name="ps_xt", bufs=1, space="PSUM")
            )
            ps_xb = ctx.enter_context(
                tc.tile_pool(name="ps_xb", bufs=2, space="PSUM")
            )
            ps_e = ctx.enter_context(
                tc.tile_pool(name="ps_e", bufs=1, space="PSUM")
            )

            def tmp(name):
                return tpool.tile([_LANES, FE], f32, tag=name, name=name)

            for ci in range(nchunks):
                r0 = ci * ROWS_CHUNK
                X = xpool.tile([_LANES, FE], f32, tag="X")
                src = x_ap[r0 : r0 + ROWS_CHUNK, :].rearrange(
                    "(b p) v -> p b v", p=128
                )
                nc.sync.dma_start(
                    X[:].rearrange("p (b v) -> p b v", b=CH), src
                )

                # --- split x = hi + lo (both bf16, exact sum) -------------- #
                XHI = xpool.tile([_LANES, FE], bf16, tag="XHI")
                nc.vector.tensor_copy(XHI[:], X[:])
                XLO = xpool.tile([_LANES, FE], bf16, tag="XLO")
                nc.vector.tensor_tensor(XLO[:], X[:], XHI[:], op=ALU.subtract)
                # --- transpose pairs into stacked XT [64, XTW] bf16 -------- #
                XT = ps_xt.tile([64, XTW], bf16, tag="XT")
                for pr in range(NPAIR):
                    nc.tensor.matmul(
                        XT[0:32, pr * 128 : (pr + 1) * 128],
                        XHI[:, pr * 32 : (pr + 1) * 32],
                        IDT[:],
                        is_transpose=True,
                        start=True,
                        stop=True,
                    )
                    nc.tensor.matmul(
                        XT[32:64, pr * 128 : (pr + 1) * 128],
                        XLO[:, pr * 32 : (pr + 1) * 32],
                        IDT[:],
                        is_transpose=True,
                        start=True,
                        stop=True,
                    )
                XTS = xts_pool.tile([64, XTW], bf16, tag="XTS")
                nc.scalar.copy(XTS[:], XT[:])

                # --- per (group, parity, half): replicate, compare --------- #
                HW2 = XTW // 2
                mtiles = {}
                for g in range(NGRP):
                    for par in range(2):
                        gp = g * 2 + par
                        for h in range(2):
                            gph = gp * 2 + h
                            XB = ps_xb.tile([GW, HW2], f32, tag="XB")
                            nc.tensor.matmul(
                                XB[:],
                                RM[:, gp * GW : (gp + 1) * GW],
                                XTS[:, h * HW2 : (h + 1) * HW2],
                                start=True,
                                stop=True,
                            )
                            M = mpool.tile([GW, HW2], f32, tag=f"M{gph}")
                            if _MASK_ON_DVE[gph]:
                                nc.vector.tensor_scalar(
                                    M[:], XB[:], THR[:, g : g + 1], None,
                                    op0=ALU.is_ge,
                                )
                            else:
                                nc.scalar.activation(
                                    M[:], XB[:], AF.Sign,
                                    bias=NTHR[:, g : g + 1],
                                )
                            mtiles[(g, par, h)] = M

                # --- stream matmuls into element-major E ------------------- #
                E = ps_e.tile([_LANES, CH * PACK], f32, tag="E")
                for b in range(CH):
                    pr, par = b // 2, b % 2
                    h, prh = pr // 4, pr % 4
                    for g in range(NGRP):
                        gph = (g * 2 + par) * 2 + h
                        M = mtiles[(g, par, h)]
                        nc.tensor.matmul(
                            E[:, b * PACK + g * 24 : b * PACK + g * 24 + 24],
                            M[:, prh * 128 : (prh + 1) * 128],
                            TT[:, gph * 24 : (gph + 1) * 24],
                            start=True,
                            stop=True,
                        )

                # --- formula (element-major [128, FE]) --------------------- #
                E3 = E[:].rearrange("p (b r) -> p b r", b=CH)

                def eap(s):
                    # stream-s view of E matching X's (b, v) column order
                    return E3[:, :, s : 96 : 6]

                XC = tmp("XC")
                nc.vector.tensor_scalar(
                    XC[:], X[:], -TAIL_BOUND, TAIL_BOUND,
                    op0=ALU.max, op1=ALU.min,
                )
                tt = tmp("tt")
                nc.vector.tensor_tensor(tt[:], XC[:], eap(0), op=ALU.subtract)
                u = tmp("u")
                nc.vector.tensor_tensor(u[:], tt[:], eap(2), op=ALU.mult)
                t3 = tmp("t3")
                nc.vector.tensor_tensor(t3[:], u[:], eap(3), op=ALU.add)
                t4 = tmp("t4")
                nc.vector.tensor_tensor(t4[:], tt[:], eap(4), op=ALU.mult)
                t5 = tmp("t5")
                nc.vector.tensor_tensor(t5[:], t4[:], eap(5), op=ALU.add)
                q = tmp("q")
                nc.gpsimd.tensor_tensor(q[:], t5[:], tt[:], op=ALU.mult)
                M_ = tmp("M_")
                nc.vector._custom_dve(
                    MULMAX, out=M_[:], in0=t3[:], in1=tt[:], s0=1e-30
                )
                D = tmp("D")
                nc.vector.tensor_scalar(D[:], q[:], 1.0, None, op0=ALU.add)
                LM = tmp("LM")
                nc.scalar.activation(LM[:], M_[:], AF.Ln)
                LD = tmp("LD")
                nc.scalar.activation(LD[:], D[:], AF.Ln)
                sdiff = tmp("sdiff")
                nc.gpsimd.tensor_tensor(sdiff[:], LM[:], LD[:], op=ALU.subtract)
                expo = tmp("expo")
                nc.scalar.activation(expo[:], sdiff[:], AF.Exp)
                outsp = tmp("outsp")
                nc.vector.tensor_tensor(outsp[:], expo[:], eap(1), op=ALU.add)
                Mp = tmp("Mp")
                nc.gpsimd.tensor_tensor(Mp[:], u[:], t3[:], op=ALU.add)
                Dp = tmp("Dp")
                nc.gpsimd.tensor_tensor(Dp[:], t4[:], t5[:], op=ALU.add)
                u1 = tmp("u1")
                nc.gpsimd.tensor_tensor(u1[:], Mp[:], D[:], op=ALU.mult)
                u2 = tmp("u2")
                nc.gpsimd.tensor_tensor(u2[:], M_[:], Dp[:], op=ALU.mult)
                P = tmp("P")
                nc.gpsimd.tensor_tensor(P[:], u1[:], u2[:], op=ALU.subtract)
                LP = tmp("LP")
                nc.scalar.activation(LP[:], P[:], AF.Ln)
                lad0 = tmp("lad0")
                nc.vector._custom_dve(
                    SUBSUB, out=lad0[:], in0=LP[:], in1=LD[:]
                )

                outs_f = opool.tile([_LANES, FE], f32, tag="outs_f")
                nc.vector._custom_dve(
                    SEL_X, out=outs_f[:], in0=X[:], in1=outsp[:],
                    s0=-TAIL_BOUND, s1=TAIL_BOUND,
                )
                lad_f = opool.tile([_LANES, FE], f32, tag="lad_f")
                nc.vector._custom_dve(
                    SEL_0, out=lad_f[:], in0=X[:], in1=lad0[:],
                    s0=-TAIL_BOUND, s1=TAIL_BOUND,
                )

                dsto = o_ap[r0 : r0 + ROWS_CHUNK, :].rearrange(
                    "(b p) v -> p b v", p=128
                )
                nc.sync.dma_start(
                    dsto, outs_f[:].rearrange("p (b v) -> p b v", b=CH)
                )
                dstl = l_ap[r0 : r0 + ROWS_CHUNK, :].rearrange(
                    "(b p) v -> p b v", p=128
                )
                nc.sync.dma_start(
                    dstl, lad_f[:].rearrange("p (b v) -> p b v", b=CH)
                )

    nc.compile()
    _PROGRAM_CACHE[key] = nc
    return nc


# --------------------------------------------------------------------------- #
# Entry point
# --------------------------------------------------------------------------- #
def _prepare(inputs, uw, uh, ud):
    inputs = np.asarray(inputs, dtype=np.float32)
    uw = np.asarray(uw, dtype=np.float32)
    uh = np.asarray(uh, dtype=np.float32)
    ud = np.asarray(ud, dtype=np.float32)
    B = inputs.shape[0]
    THR, NTHR, Rm, T = _build_tables(uw, uh, ud)
    import ml_dtypes
    Rm = Rm.astype(ml_dtypes.bfloat16)
    ident = np.eye(_LANES, dtype=ml_dtypes.bfloat16)

    rows_per_core = -(-B // NCORES)
    rows_per_core = ((rows_per_core + ROWS_CHUNK - 1) // ROWS_CHUNK) * ROWS_CHUNK
    Bp = rows_per_core * NCORES
    xp = np.zeros((Bp, V), dtype=np.float32)
    xp[:B] = inputs

    nc = _build_program(rows_per_core)
    in_maps = []
    for c in range(NCORES):
        xc = xp[c * rows_per_core : (c + 1) * rows_per_core]
        in_maps.append(
            {"x": xc, "thr": THR, "nthr": NTHR, "rmat": Rm, "tbl": T,
             "ident": ident}
        )
    return nc, in_maps, B, Bp, rows_per_core


def kernel(inputs, unnormalized_widths, unnormalized_heights,
           unnormalized_derivatives):
    nc, in_maps, B, Bp, rows_per_core = _prepare(
        inputs, unnormalized_widths, unnormalized_heights,
        unnormalized_derivatives,
    )
    from concourse.bass_utils import run_bass_kernel_spmd

    res = run_bass_kernel_spmd(nc, in_maps, core_ids=list(range(NCORES)))

    outs = np.empty((Bp, V), dtype=np.float32)
    lads = np.empty((Bp, V), dtype=np.float32)
    for c in range(NCORES):
        r = res.results[c]
        outs[c * rows_per_core : (c + 1) * rows_per_core] = r["out"]
        lads[c * rows_per_core : (c + 1) * rows_per_core] = r["lad"]
    return outs[:B], lads[:B]


def run_traced(inputs_dict):
    """Run once with tracing; returns HW exec time in ns (or None)."""
    nc, in_maps, B, Bp, rows_per_core = _prepare(
        inputs_dict["inputs"],
        inputs_dict["unnormalized_widths"],
        inputs_dict["unnormalized_heights"],
        inputs_dict["unnormalized_derivatives"],
    )
    from concourse.bass_utils import run_bass_kernel_spmd

    res = run_bass_kernel_spmd(
        nc, in_maps, core_ids=list(range(NCORES)), trace=True
    )
    return res.exec_time_ns


if __name__ == "__main__":
    B = 4096
    rng = np.random.default_rng(0)
    x = rng.standard_normal((B, V)).astype(np.float32)
    uw = rng.random((V, K), dtype=np.float32)
    uh = rng.random((V, K), dtype=np.float32)
    ud = rng.random((V, K - 1), dtype=np.float32)
    o, l = kernel(x, uw, uh, ud)
    print("kernel ran", o.shape, l.shape)


# revision 10
# speedup vs baseline: 2.8074x; 1.2383x over previous
"""Rational-quadratic spline (neural spline flow) forward kernel for TRN2.

Architecture (v2 — "knots on partitions" one-hot/step matmul):

  - Data-parallel over 8 NeuronCores, batch rows sharded (62720 rows/core).
  - Per chunk of 14 row-blocks (1792 rows x 16 vars = 28672 elements):
      1. DMA x in element-major [128, (block, var)].
      2. PE transposes row-block PAIRS [128, 32] -> XT PSUM [32, pair*128]
         (f32r, exact), one DVE/ACT copy escapes XT to SBUF.
      3. PE "replication" matmuls R_gp^T @ XTS -> XB [120, cols] per
         (4-var group g, block parity): partition r = 4k+c holds x of var
         4g+c replicated over the 30 knot rows k.
      4. One compare op per (g, parity) produces ALL 29 step masks at once
         (DVE is_ge -> {0,1}, or ACT Sign -> {-1,1}); knot row k=29 has
         threshold -1e30 == always-on and carries the stream base.
      5. One small transpose-matmul per (block, group): M^T @ T_g -> E
         [elements, 4 vars x 6 streams] in PSUM: all six telescoped
         stream sums {cw, ch, AM, BM, AD, BD} per element in one shot.
      6. Rational-quadratic formula element-major across DVE/Pool/ACT;
         division via exp(ln M - ln D); outside [-5,5] select(x)/select(0).
"""

import numpy as np

TAIL_BOUND = 5.0
MIN_BIN_WIDTH = 1e-3
MIN_BIN_HEIGHT = 1e-3
MIN_DERIVATIVE = 1e-3
K = 30
V = 16
NCORES = 8

_LANES = 128
CH = 16                 # row-blocks per chunk
ROWS_CHUNK = CH * 128   # 2048
PACK = 128              # E columns per block (4 groups x 24, padded to a
                        # quarter PSUM bank so no matmul straddles a bank)
NGRP = 4                # 4-variable groups
GW = 120                # partitions per XB/mask tile (30 knots x 4 vars)
# mask producer per (g, parity, half): index = (g*2+par)*2+h; True -> DVE is_ge
_MASK_ON_DVE = [True, False, False, True, False, True, False, True,
                True, False, True, False, False, True, False, True]


# --------------------------------------------------------------------------- #
# Custom DVE ops
# --------------------------------------------------------------------------- #
_OPS_REGISTERED = {}


def _register_custom_ops():
    if _OPS_REGISTERED:
        return _OPS_REGISTERED
    import concourse.dve_ops as dve_ops
    from concourse.dve_ops import DveOp, has_src1
    from concourse.dve_spec import Spec, Src0, Src1, C0, C1, Zero, select, maxx, lower
    from concourse.dve_uop import DveOpSpec

    def mk(name, spec):
        sha = {}
        for ver in ("v3", "v4"):
            compiled = DveOpSpec(
                name=name, uops=lower(spec, ver=ver), rd1_en=has_src1(spec)
            )
            sha[ver] = compiled.sha(ver)
        op = DveOp(name, spec, subdim=False, uops_sha=sha)
        dve_ops.OPS.append(op)
        dve_ops.CUSTOM_DVE_SPECS[op.name] = op.spec
        dve_ops._SUB_OPCODE_FOR_NAME[op.name] = (
            dve_ops._CUSTOM_DVE_ROW_BASE + len(dve_ops.OPS) - 1
        )
        assert dve_ops._SUB_OPCODE_FOR_NAME[op.name] < 0x20
        return op

    MULMAX = mk(
        "RQS2_MULMAX_ANT",
        Spec(
            body=maxx(Src0 * Src1, C0),
            reference=lambda in0, in1, s0, s1, imm2: np.maximum(
                in0 * in1, s0
            ).astype(np.float32),
        ),
    )
    SUBSUB = mk(
        "RQS2_SUBSUB_ANT",
        Spec(
            body=(Src0 - Src1) - Src1,
            reference=lambda in0, in1, s0, s1, imm2: (in0 - 2.0 * in1).astype(
                np.float32
            ),
        ),
    )
    SEL_X = mk(
        "RQS2_SEL_X_ANT",
        Spec(
            body=select((Src0 >= C0) & (Src0 <= C1), Src1, Src0),
            reference=lambda in0, in1, s0, s1, imm2: np.where(
                (in0 >= s0) & (in0 <= s1), in1, in0
            ).astype(np.float32),
        ),
    )
    SEL_0 = mk(
        "RQS2_SEL_0_ANT",
        Spec(
            body=select((Src0 >= C0) & (Src0 <= C1), Src1, Zero),
            reference=lambda in0, in1, s0, s1, imm2: np.where(
                (in0 >= s0) & (in0 <= s1), in1, 0.0
            ).astype(np.float32),
        ),
    )
    _OPS_REGISTERED.update(MULMAX=MULMAX, SUBSUB=SUBSUB, SEL_X=SEL_X, SEL_0=SEL_0)
    return _OPS_REGISTERED


# --------------------------------------------------------------------------- #
# Host-side table construction
# --------------------------------------------------------------------------- #
def _softmax(x, axis=-1):
    x = x - x.max(axis=axis, keepdims=True)
    e = np.exp(x)
    return e / e.sum(axis=axis, keepdims=True)


def _softplus(x):
    return np.log1p(np.exp(-np.abs(x))) + np.maximum(x, 0)


def _knots(unnorm, min_bin, lo, hi):
    w = _softmax(unnorm.astype(np.float64), axis=-1)
    w = min_bin + (1.0 - min_bin * K) * w
    cw = np.cumsum(w, axis=-1)
    cw = np.pad(cw, ((0, 0), (1, 0)))
    cw = (hi - lo) * cw + lo
    cw[..., 0] = lo
    cw[..., -1] = hi
    return cw  # (V, K+1)


def _build_tables(uw, uh, ud):
    """Returns THR (120,4), NTHR (120,4), R (64, 8*120), T (120, 16*24)."""
    lo, hi = -TAIL_BOUND, TAIL_BOUND
    const = np.log(np.exp(1.0 - MIN_DERIVATIVE) - 1.0)
    udp = np.concatenate(
        [np.full((V, 1), const), ud.astype(np.float64), np.full((V, 1), const)],
        axis=-1,
    )
    d = MIN_DERIVATIVE + _softplus(udp)  # (V,K+1)

    cw = _knots(uw, MIN_BIN_WIDTH, lo, hi)
    chts = _knots(uh, MIN_BIN_HEIGHT, lo, hi)

    w = cw[:, 1:] - cw[:, :-1]
    h = chts[:, 1:] - chts[:, :-1]
    delta = h / w
    a = 1.0 / w
    dk = d[:, :-1]
    dk1 = d[:, 1:]

    AM = h * a * a * (1.0 - dk / delta)
    BM = h * a * dk / delta
    gam = (dk + dk1 - 2.0 * delta) / delta
    AD = -gam * a * a
    BD = gam * a

    streams = [cw[:, :-1], chts[:, :-1], AM, BM, AD, BD]  # each (V,K)
    thr = cw[:, 1:K]  # (V,29) interior knots

    THR = np.full((GW, NGRP), -1e30, dtype=np.float32)
    T = np.zeros((GW, 16 * 24), dtype=np.float32)
    Rm = np.zeros((64, 8 * GW), dtype=np.float32)
    for g in range(NGRP):
        for par in range(2):
            gp = g * 2 + par
            for c in range(4):
                v = 4 * g + c
                Rm[16 * par + v, gp * GW + np.arange(30) * 4 + c] = 1.0
                Rm[32 + 16 * par + v, gp * GW + np.arange(30) * 4 + c] = 1.0
                if par == 0:
                    for k in range(29):
                        THR[4 * k + c, g] = thr[v, k]
            for h in range(2):
                gph = gp * 2 + h
                on_dve = _MASK_ON_DVE[gph]
                for c in range(4):
                    v = 4 * g + c
                    for si, S in enumerate(streams):
                        dS = np.diff(S[v])  # (29,)
                        base = S[v, 0]
                        col = gph * 24 + c * 6 + si
                        if on_dve:
                            T[4 * np.arange(29) + c, col] = dS.astype(np.float32)
                            T[4 * 29 + c, col] = np.float32(base)
                        else:
                            T[4 * np.arange(29) + c, col] = (dS / 2.0).astype(
                                np.float32
                            )
                            T[4 * 29 + c, col] = np.float32(
                                base + dS.sum() / 2.0
                            )
    NTHR = (-THR).astype(np.float32)
    return THR, NTHR, Rm, T


# --------------------------------------------------------------------------- #
# Bass program
# --------------------------------------------------------------------------- #
_PROGRAM_CACHE = {}


def _build_program(rows_per_core):
    key = rows_per_core
    if key in _PROGRAM_CACHE:
        return _PROGRAM_CACHE[key]

    import concourse.bass as bass
    import concourse.bacc as bacc
    import concourse.tile as tile
    from concourse import mybir
    from contextlib import ExitStack

    ops = _register_custom_ops()
    MULMAX, SUBSUB = ops["MULMAX"], ops["SUBSUB"]
    SEL_X, SEL_0 = ops["SEL_X"], ops["SEL_0"]

    f32 = mybir.dt.float32
    f32r = mybir.dt.float32r
    bf16 = mybir.dt.bfloat16
    ALU = mybir.AluOpType
    AF = mybir.ActivationFunctionType

    assert rows_per_core % ROWS_CHUNK == 0
    nchunks = rows_per_core // ROWS_CHUNK
    FE = CH * 16          # element-major free size per chunk (224)
    NPAIR = CH // 2       # 7
    XTW = NPAIR * 128     # 896 cols per parity

    nc = bacc.Bacc(
        "TRN2", target_bir_lowering=False, debug=False, num_devices=NCORES
    )
    x_d = nc.dram_tensor("x", (rows_per_core, 16), f32, kind="ExternalInput")
    thr_d = nc.dram_tensor("thr", (GW, NGRP), f32, kind="ExternalInput")
    nthr_d = nc.dram_tensor("nthr", (GW, NGRP), f32, kind="ExternalInput")
    r_d = nc.dram_tensor("rmat", (64, 8 * GW), bf16, kind="ExternalInput")
    t_d = nc.dram_tensor("tbl", (GW, 16 * 24), f32, kind="ExternalInput")
    i_d = nc.dram_tensor("ident", (_LANES, _LANES), bf16, kind="ExternalInput")
    o_d = nc.dram_tensor("out", (rows_per_core, 16), f32, kind="ExternalOutput")
    l_d = nc.dram_tensor("lad", (rows_per_core, 16), f32, kind="ExternalOutput")

    x_ap, o_ap, l_ap = x_d.ap(), o_d.ap(), l_d.ap()

    with tile.TileContext(nc) as tc:
        with ExitStack() as ctx:
            cpool = ctx.enter_context(tc.tile_pool(name="const", bufs=1))
            THR = cpool.tile([GW, NGRP], f32)
            nc.sync.dma_start(THR[:], thr_d.ap())
            NTHR = cpool.tile([GW, NGRP], f32)
            nc.sync.dma_start(NTHR[:], nthr_d.ap())
            RM = cpool.tile([64, 8 * GW], bf16)
            nc.sync.dma_start(RM[:], r_d.ap())
            TT = cpool.tile([GW, 16 * 24], f32)
            nc.sync.dma_start(TT[:], t_d.ap())
            IDT = cpool.tile([_LANES, _LANES], bf16)
            nc.sync.dma_start(IDT[:], i_d.ap())

            xpool = ctx.enter_context(tc.tile_pool(name="xin", bufs=2))
            xts_pool = ctx.enter_context(tc.tile_pool(name="xts", bufs=2))
            mpool = ctx.enter_context(tc.tile_pool(name="masks", bufs=2))
            tpool = ctx.enter_context(tc.tile_pool(name="tmp", bufs=1))
            opool = ctx.enter_context(tc.tile_pool(name="outs", bufs=2))
            ps_xt = ctx.enter_context(
                tc.tile_pool(name="ps_xt", bufs=2, space="PSUM")
            )
            ps_xb = ctx.enter_context(
                tc.tile_pool(name="ps_xb", bufs=2, space="PSUM")
            )
            ps_e = ctx.enter_context(
                tc.tile_pool(name="ps_e", bufs=1, space="PSUM")
            )

            def tmp(name):
                return tpool.tile([_LANES, FE], f32, tag=name, name=name)

            for ci in range(nchunks):
                r0 = ci * ROWS_CHUNK
                X = xpool.tile([_LANES, FE], f32, tag="X")
                src = x_ap[r0 : r0 + ROWS_CHUNK, :].rearrange(
                    "(b p) v -> p b v", p=128
                )
                nc.sync.dma_start(
                    X[:].rearrange("p (b v) -> p b v", b=CH), src
                )

                # --- split x = hi + lo (both bf16, exact sum) -------------- #
                XHI = xpool.tile([_LANES, FE], bf16, tag="XHI")
                nc.vector.tensor_copy(XHI[:], X[:])
                XLO = xpool.tile([_LANES, FE], bf16, tag="XLO")
                nc.vector.tensor_tensor(XLO[:], X[:], XHI[:], op=ALU.subtract)
                # --- transpose pairs into stacked XT [64, XTW] bf16 -------- #
                XT = ps_xt.tile([64, XTW], bf16, tag="XT")
                for pr in range(NPAIR):
                    nc.tensor.matmul(
                        XT[0:32, pr * 128 : (pr + 1) * 128],
                        XHI[:, pr * 32 : (pr + 1) * 32],
                        IDT[:],
                        is_transpose=True,
                        start=True,
                        stop=True,
                    )
                    nc.tensor.matmul(
                        XT[32:64, pr * 128 : (pr + 1) * 128],
                        XLO[:, pr * 32 : (pr + 1) * 32],
                        IDT[:],
                        is_transpose=True,
                        start=True,
                        stop=True,
                    )
                XTS = xts_pool.tile([64, XTW], bf16, tag="XTS")
                nc.scalar.copy(XTS[:], XT[:])

                # --- per (group, parity, half): replicate, compare --------- #
                HW2 = XTW // 2
                mtiles = {}
                for g in range(NGRP):
                    for par in range(2):
                        gp = g * 2 + par
                        for h in range(2):
                            gph = gp * 2 + h
                            XB = ps_xb.tile([GW, HW2], f32, tag="XB")
                            nc.tensor.matmul(
                                XB[:],
                                RM[:, gp * GW : (gp + 1) * GW],
                                XTS[:, h * HW2 : (h + 1) * HW2],
                                start=True,
                                stop=True,
                            )
                            M = mpool.tile([GW, HW2], f32, tag=f"M{gph}")
                            if _MASK_ON_DVE[gph]:
                                nc.vector.tensor_scalar(
                                    M[:], XB[:], THR[:, g : g + 1], None,
                                    op0=ALU.is_ge,
                                )
                            else:
                                nc.scalar.activation(
                                    M[:], XB[:], AF.Sign,
                                    bias=NTHR[:, g : g + 1],
                                )
                            mtiles[(g, par, h)] = M

                # --- stream matmuls into element-major E ------------------- #
                E = ps_e.tile([_LANES, CH * PACK], f32, tag="E")
                for b in range(CH):
                    pr, par = b // 2, b % 2
                    h, prh = pr // 4, pr % 4
                    for g in range(NGRP):
                        gph = (g * 2 + par) * 2 + h
                        M = mtiles[(g, par, h)]
                        nc.tensor.matmul(
                            E[:, b * PACK + g * 24 : b * PACK + g * 24 + 24],
                            M[:, prh * 128 : (prh + 1) * 128],
                            TT[:, gph * 24 : (gph + 1) * 24],
                            start=True,
                            stop=True,
                        )

                # --- formula (element-major [128, FE]) --------------------- #
                E3 = E[:].rearrange("p (b r) -> p b r", b=CH)

                def eap(s):
                    # stream-s view of E matching X's (b, v) column order
                    return E3[:, :, s : 96 : 6]

                XC = tmp("XC")
                nc.gpsimd.tensor_scalar(
                    XC[:], X[:], -TAIL_BOUND, TAIL_BOUND,
                    op0=ALU.max, op1=ALU.min,
                )
                tt = tmp("tt")
                nc.vector.tensor_tensor(tt[:], XC[:], eap(0), op=ALU.subtract)
                u = tmp("u")
                nc.vector.tensor_tensor(u[:], tt[:], eap(2), op=ALU.mult)
                t3 = tmp("t3")
                nc.vector.tensor_tensor(t3[:], u[:], eap(3), op=ALU.add)
                t4 = tmp("t4")
                nc.vector.tensor_tensor(t4[:], tt[:], eap(4), op=ALU.mult)
                t5 = tmp("t5")
                nc.vector.tensor_tensor(t5[:], t4[:], eap(5), op=ALU.add)
                q = tmp("q")
                nc.gpsimd.tensor_tensor(q[:], t5[:], tt[:], op=ALU.mult)
                D = tmp("D")
                nc.gpsimd.tensor_scalar(D[:], q[:], 1.0, None, op0=ALU.add)
                # numerator fold: out = (M + ch*D)/D, so E_ch is read early
                chD = tmp("chD")
                nc.vector.tensor_tensor(chD[:], D[:], eap(1), op=ALU.mult)
                M_ = tmp("M_")
                nc.vector._custom_dve(
                    MULMAX, out=M_[:], in0=t3[:], in1=tt[:], s0=1e-30
                )
                N_ = tmp("N_")
                nc.gpsimd.tensor_tensor(N_[:], M_[:], chD[:], op=ALU.add)
                rD = tmp("rD")
                nc.vector.reciprocal(rD[:], D[:])
                outsp = tmp("outsp")
                nc.gpsimd.tensor_tensor(outsp[:], N_[:], rD[:], op=ALU.mult)
                LD = tmp("LD")
                nc.scalar.activation(LD[:], D[:], AF.Ln)
                Mp = tmp("Mp")
                nc.gpsimd.tensor_tensor(Mp[:], u[:], t3[:], op=ALU.add)
                Dp = tmp("Dp")
                nc.gpsimd.tensor_tensor(Dp[:], t4[:], t5[:], op=ALU.add)
                u1 = tmp("u1")
                nc.gpsimd.tensor_tensor(u1[:], Mp[:], D[:], op=ALU.mult)
                u2 = tmp("u2")
                nc.gpsimd.tensor_tensor(u2[:], M_[:], Dp[:], op=ALU.mult)
                P = tmp("P")
                nc.gpsimd.tensor_tensor(P[:], u1[:], u2[:], op=ALU.subtract)
                LP = tmp("LP")
                nc.scalar.activation(LP[:], P[:], AF.Ln)
                lad0 = tmp("lad0")
                nc.vector._custom_dve(
                    SUBSUB, out=lad0[:], in0=LP[:], in1=LD[:]
                )

                outs_f = opool.tile([_LANES, FE], f32, tag="outs_f")
                nc.vector._custom_dve(
                    SEL_X, out=outs_f[:], in0=X[:], in1=outsp[:],
                    s0=-TAIL_BOUND, s1=TAIL_BOUND,
                )
                lad_f = opool.tile([_LANES, FE], f32, tag="lad_f")
                nc.vector._custom_dve(
                    SEL_0, out=lad_f[:], in0=X[:], in1=lad0[:],
                    s0=-TAIL_BOUND, s1=TAIL_BOUND,
                )

                dsto = o_ap[r0 : r0 + ROWS_CHUNK, :].rearrange(
                    "(b p) v -> p b v", p=128
                )
                nc.sync.dma_start(
                    dsto, outs_f[:].rearrange("p (b v) -> p b v", b=CH)
                )
                dstl = l_ap[r0 : r0 + ROWS_CHUNK, :].rearrange(
                    "(b p) v -> p b v", p=128
                )
                nc.sync.dma_start(
                    dstl, lad_f[:].rearrange("p (b v) -> p b v", b=CH)
                )

    nc.compile()
    _PROGRAM_CACHE[key] = nc
    return nc


# --------------------------------------------------------------------------- #
# Entry point
# --------------------------------------------------------------------------- #
def _prepare(inputs, uw, uh, ud):
    inputs = np.asarray(inputs, dtype=np.float32)
    uw = np.asarray(uw, dtype=np.float32)
    uh = np.asarray(uh, dtype=np.float32)
    ud = np.asarray(ud, dtype=np.float32)
    B = inputs.shape[0]
    THR, NTHR, Rm, T = _build_tables(uw, uh, ud)
    import ml_dtypes
    Rm = Rm.astype(ml_dtypes.bfloat16)
    ident = np.eye(_LANES, dtype=ml_dtypes.bfloat16)

    rows_per_core = -(-B // NCORES)
    rows_per_core = ((rows_per_core + ROWS_CHUNK - 1) // ROWS_CHUNK) * ROWS_CHUNK
    Bp = rows_per_core * NCORES
    xp = np.zeros((Bp, V), dtype=np.float32)
    xp[:B] = inputs

    nc = _build_program(rows_per_core)
    in_maps = []
    for c in range(NCORES):
        xc = xp[c * rows_per_core : (c + 1) * rows_per_core]
        in_maps.append(
            {"x": xc, "thr": THR, "nthr": NTHR, "rmat": Rm, "tbl": T,
             "ident": ident}
        )
    return nc, in_maps, B, Bp, rows_per_core


def kernel(inputs, unnormalized_widths, unnormalized_heights,
           unnormalized_derivatives):
    nc, in_maps, B, Bp, rows_per_core = _prepare(
        inputs, unnormalized_widths, unnormalized_heights,
        unnormalized_derivatives,
    )
    from concourse.bass_utils import run_bass_kernel_spmd

    res = run_bass_kernel_spmd(nc, in_maps, core_ids=list(range(NCORES)))

    outs = np.empty((Bp, V), dtype=np.float32)
    lads = np.empty((Bp, V), dtype=np.float32)
    for c in range(NCORES):
        r = res.results[c]
        outs[c * rows_per_core : (c + 1) * rows_per_core] = r["out"]
        lads[c * rows_per_core : (c + 1) * rows_per_core] = r["lad"]
    return outs[:B], lads[:B]


def run_traced(inputs_dict):
    """Run once with tracing; returns HW exec time in ns (or None)."""
    nc, in_maps, B, Bp, rows_per_core = _prepare(
        inputs_dict["inputs"],
        inputs_dict["unnormalized_widths"],
        inputs_dict["unnormalized_heights"],
        inputs_dict["unnormalized_derivatives"],
    )
    from concourse.bass_utils import run_bass_kernel_spmd

    res = run_bass_kernel_spmd(
        nc, in_maps, core_ids=list(range(NCORES)), trace=True
    )
    return res.exec_time_ns


if __name__ == "__main__":
    B = 4096
    rng = np.random.default_rng(0)
    x = rng.standard_normal((B, V)).astype(np.float32)
    uw = rng.random((V, K), dtype=np.float32)
    uh = rng.random((V, K), dtype=np.float32)
    ud = rng.random((V, K - 1), dtype=np.float32)
    o, l = kernel(x, uw, uh, ud)
    print("kernel ran", o.shape, l.shape)
